# revision 1
# baseline (speedup 1.0000x reference)
"""Trainium2 Bass kernel for nn_GATSubstAttention (GAT with substructure attention).

8 NeuronCores, SPMD. Nodes dst-sharded into 8 contiguous ranges; edges sorted
by dst. Per-edge data fetched with batched indirect-DMA row gathers. Segment
softmax + weighted aggregation via alpha-folded one-hot matmuls accumulated in
PSUM over fixed dst windows (32 for layer 1 x 4 heads, 128 for layer 2). Raw
65-wide features are aggregated; the W1 projection happens per-window after
aggregation. The inter-layer node table is exchanged with AllGather;
substructure stats and graph pooling use AllReduce.
"""

import sys

sys.path.insert(0, "/opt/trn_rl_repo")

import numpy as np

import concourse.bass as bass
import concourse.mybir as mybir
from concourse.tile import TileContext, add_dep_helper

F32 = mybir.dt.float32
I32 = mybir.dt.int32
AX = mybir.AluOpType
AF = mybir.ActivationFunctionType

NC = 8
FEAT = 64
HID = 64
HEADS = 4
NSUB = 32
NG = 128   # num graphs
W1WIN = 32
W2WIN = 128
GB = 32    # tiles per gather batch
BGW = 76   # big1 table row width (f32): [x(64), aw, 1, a_s(4), a_d(4), pad(2)]
XCW = 68   # xchg row width (f32): [m(64), 1, a_s2, a_d2, pad]
RHW = 70   # layer-1 gathered row: [x(64), aw, 1, a_s(4)]


# ----------------------------------------------------------------------------
# Host-side preparation (indexing / layout only, no arithmetic on values)
# ----------------------------------------------------------------------------

def _prep(x, edge_index, batch):
    N = x.shape[0]
    NSH = -(-N // NC)
    NSHP = -(-NSH // 128) * 128
    NW1 = NSHP // W1WIN
    NW2 = NSHP // W2WIN

    src = np.concatenate([edge_index[0], np.arange(N, dtype=np.int64)])
    dst = np.concatenate([edge_index[1], np.arange(N, dtype=np.int64)])
    owner = dst // NSH
    pack = lambda n: (n // NSH) * NSHP + (n % NSH)

    per_core = []
    cnt1 = np.zeros((NC, NW1), np.int64)
    cnt2 = np.zeros((NC, NW2), np.int64)
    for c in range(NC):
        n0 = c * NSH
        n1 = min(n0 + NSH, N)
        sel = np.nonzero(owner == c)[0]
        s = src[sel]
        dl = dst[sel] - n0
        # fake edges so padded dst rows have finite denominators
        npad = NSHP - (n1 - n0)
        if npad:
            s = np.concatenate([s, np.full(npad, n0, np.int64)])
            dl = np.concatenate([dl, np.arange(n1 - n0, NSHP, dtype=np.int64)])
        order = np.argsort(dl, kind="stable")
        s, dl = s[order], dl[order]
        per_core.append((pack(s).astype(np.int32), dl))
        cnt1[c] = np.bincount(dl // W1WIN, minlength=NW1)
        cnt2[c] = np.bincount(dl // W2WIN, minlength=NW2)

    tiles1 = np.maximum(-(-cnt1.max(axis=0) // 128), 1)
    tiles2 = np.maximum(-(-cnt2.max(axis=0) // 128), 1)
    T1, T2 = int(tiles1.sum()), int(tiles2.sum())

    def layout(win, ntiles, T, c):
        sp, dl = per_core[c]
        e_src = np.zeros((T, 128), np.int32)
        e_dst = np.zeros((T, 128), np.int32)
        doff = np.full((T, 128), -1.0, np.float32)
        w = dl // win
        nw = len(ntiles)
        starts = np.concatenate([[0], np.cumsum(ntiles)]).astype(int)
        wstart = np.searchsorted(w, np.arange(nw))
        wend = np.searchsorted(w, np.arange(nw) + 1)
        cNSHP = c * NSHP
        for wi in range(nw):
            a, b = int(wstart[wi]), int(wend[wi])
            n = b - a
            nt = int(ntiles[wi])
            t0 = starts[wi]
            flat_s = np.zeros(nt * 128, np.int32)
            flat_d = np.full(nt * 128, cNSHP, np.int64)
            flat_o = np.full(nt * 128, -1.0, np.float32)
            if n:
                flat_s[:n] = sp[a:b]
                flat_d[:n] = cNSHP + dl[a:b]
                flat_o[:n] = (dl[a:b] - wi * win).astype(np.float32)
            e_src[t0:t0 + nt] = flat_s.reshape(-1, 128)
            e_dst[t0:t0 + nt] = flat_d.astype(np.int32).reshape(-1, 128)
            doff[t0:t0 + nt] = flat_o.reshape(-1, 128)
        return e_src, e_dst, doff

    def to_batches(arr, T):
        NB = -(-T // GB)
        pad = NB * GB - T
        if pad:
            if arr.dtype == np.int32:
                fill = np.zeros((pad, 128), arr.dtype)
            else:
                fill = np.full((pad, 128), -1.0, arr.dtype)
            arr = np.concatenate([arr, fill])
        return np.ascontiguousarray(
            arr.reshape(-1, GB, 128).transpose(0, 2, 1))

    b1 = [[], [], []]
    b2 = [[], [], []]
    for c in range(NC):
        for i, a in enumerate(layout(W1WIN, tiles1, T1, c)):
            if i == 1:  # dst index -> flat element index of a_d in big1
                a = a * np.int32(BGW) + np.int32(FEAT + 6)
            b1[i].append(to_batches(a, T1))
        for i, a in enumerate(layout(W2WIN, tiles2, T2, c)):
            if i == 1:  # dst index -> flat element index of a_d2 in xchg
                a = a * np.int32(XCW) + np.int32(HID + 2)
            b2[i].append(to_batches(a, T2))

    xs = np.zeros((NC, NSHP, FEAT), np.float32)
    boff = np.full((NC, NSHP), -1.0, np.float32)
    for c in range(NC):
        n0, n1 = c * NSH, min((c + 1) * NSH, N)
        xs[c, :n1 - n0] = x[n0:n1]
        xs[c, n1 - n0:, 5] = -1.0
        boff[c, :n1 - n0] = batch[n0:n1].astype(np.float32)
    xTs = np.ascontiguousarray(xs.transpose(0, 2, 1))

    return dict(
        N=N, NSH=NSH, NSHP=NSHP, NW1=NW1, NW2=NW2,
        tiles1=[int(v) for v in tiles1], tiles2=[int(v) for v in tiles2],
        T1=T1, T2=T2,
        b1s=np.stack(b1[0]), b1d=np.stack(b1[1]), b1o=np.stack(b1[2]),
        b2s=np.stack(b2[0]), b2d=np.stack(b2[1]), b2o=np.stack(b2[2]),
        xs=xs, xTs=xTs,
        boff=boff.reshape(NC, NW2, W2WIN),
    )


def _weights_pack(w):
    """Pure layout packing of the weight arrays."""
    W1 = np.asarray(w["W1"], np.float32)        # [65, 256]
    W2 = np.asarray(w["W2"], np.float32)        # [256, 64]
    att_s1 = np.asarray(w["att_s1"], np.float32)
    att_d1 = np.asarray(w["att_d1"], np.float32)
    A1 = np.zeros((HEADS * HID, 2 * HEADS), np.float32)
    for h in range(HEADS):
        A1[h * HID:(h + 1) * HID, h] = att_s1[h]
        A1[h * HID:(h + 1) * HID, HEADS + h] = att_d1[h]
    return dict(
        A1=A1,
        W1T=np.ascontiguousarray(W1.T),                       # [256, 65]
        W1ext=np.concatenate(                                 # [66, 256]
            [W1, np.asarray(w["b1"], np.float32)[None, :]], axis=0),
        W2=W2,
        att2=np.stack([np.asarray(w["att_s2"], np.float32)[0],
                       np.asarray(w["att_d2"], np.float32)[0]], axis=1),
        wsa1e=np.concatenate([np.asarray(w["w_sa1"], np.float32),
                              np.asarray(w["b_sa1"], np.float32)[None, :]], axis=0),
        wsa2e=np.concatenate([np.asarray(w["w_sa2"], np.float32),
                              np.asarray(w["b_sa2"], np.float32)[None, :]], axis=0),
        Wp1e=np.concatenate([np.asarray(w["Wp1"], np.float32),
                             np.asarray(w["bp1"], np.float32)[None, :]], axis=0),
        Wp2=np.asarray(w["Wp2"], np.float32),
        bp2=np.asarray(w["bp2"], np.float32).reshape(1, 1),
        b2row=np.asarray(w["b2"], np.float32).reshape(1, HID),
    )


# ----------------------------------------------------------------------------
# Device program (identical on all 8 cores; per-core data differs)
# ----------------------------------------------------------------------------

def _build(cfg):
    NSHP, NW1, NW2 = cfg["NSHP"], cfg["NW1"], cfg["NW2"]
    tiles1, tiles2 = cfg["tiles1"], cfg["tiles2"]
    T1, T2 = cfg["T1"], cfg["T2"]
    NB1 = -(-T1 // GB)
    NB2 = -(-T2 // GB)
    NTOT = NC * NSHP
    NNT = NSHP // 128
    NCH = NSHP // 128  # layer-1 chunks (4 windows of 32 = 128 dst)

    nc = bass.Bass()
    P = lambda name, shape, dt=F32: nc.declare_dram_parameter(
        name, shape, dt, isOutput=False)

    xs = P("xs", [NSHP, FEAT])
    xTs = P("xTs", [FEAT, NSHP])
    e1s = P("e1s", [NB1, 128, GB], I32)
    e1d = P("e1d", [NB1, 128, GB], I32)
    e1o = P("e1o", [NB1, 128, GB])
    e2s = P("e2s", [NB2, 128, GB], I32)
    e2d = P("e2d", [NB2, 128, GB], I32)
    e2o = P("e2o", [NB2, 128, GB])
    boffp = P("boff", [NW2, W2WIN])
    A1p = P("A1", [HEADS * HID, 2 * HEADS])
    W1Tp = P("W1T", [HEADS * HID, FEAT + 1])
    W1extp = P("W1ext", [FEAT + 2, HEADS * HID])
    W2p = P("W2", [HEADS * HID, HID])
    att2p = P("att2", [HID, 2])
    wsa1ep = P("wsa1e", [FEAT + 1, NSUB])
    wsa2ep = P("wsa2e", [NSUB + 1, 1])
    Wp1ep = P("Wp1e", [HID + 1, HID // 2])
    Wp2p = P("Wp2", [HID // 2, 1])
    bp2p = P("bp2", [1, 1])
    b2rowp = P("b2row", [1, HID])
    identp = P("ident", [128, 128])
    constp = P("consts", [1, 3 * 128])
    outp = nc.declare_dram_parameter("out", [NG, 1], F32, isOutput=True)
    dbg = cfg.get("dbg")
    if dbg:
        dbg_ssum = nc.declare_dram_parameter("dbg_ssum", [NSUB, FEAT + 1], F32, isOutput=True)
        dbg_wrow = nc.declare_dram_parameter("dbg_wrow", [1, NSUB], F32, isOutput=True)
        dbg_small = nc.declare_dram_parameter("dbg_small", [256, 10], F32, isOutput=True)
        dbg_agg = nc.declare_dram_parameter("dbg_agg", [128, FEAT + 2], F32, isOutput=True)
        dbg_stg = nc.declare_dram_parameter("dbg_stg", [HID, HEADS * 128], F32, isOutput=True)
        dbg_xchg = nc.declare_dram_parameter("dbg_xchg", [256, XCW], F32, isOutput=True)
        dbg_g = nc.declare_dram_parameter("dbg_g", [NG, HID + 1], F32, isOutput=True)
        dbg_psw = nc.declare_dram_parameter("dbg_psw", [128, FEAT + 2], F32, isOutput=True)
        dbg_exb = nc.declare_dram_parameter("dbg_exb", [128, GB * HEADS], F32, isOutput=True)
        dbg_grh = nc.declare_dram_parameter("dbg_grh", [128, GB * RHW], F32, isOutput=True)
        dbg_gd = nc.declare_dram_parameter("dbg_gd", [128, GB * HEADS], F32, isOutput=True)
        dbg_e4 = nc.declare_dram_parameter("dbg_e4", [128, GB * HEADS], F32, isOutput=True)

    big1_l = nc.dram_tensor("big1_l", [NSHP, BGW], F32)
    big1 = nc.dram_tensor("big1", [NTOT, BGW], F32)
    xchg_l = nc.dram_tensor("xchg_l", [NSHP, XCW], F32)
    xchg = nc.dram_tensor("xchg", [NTOT, XCW], F32)
    ssum_l = nc.dram_tensor("ssum_l", [NSUB, FEAT + 1], F32)
    ssum_g = nc.dram_tensor("ssum_g", [NSUB, FEAT + 1], F32)
    g_l = nc.dram_tensor("g_l", [NG, HID + 1], F32)
    g_g = nc.dram_tensor("g_g", [NG, HID + 1], F32)

    RG = [list(range(NC))]

    with TileContext(nc) as tc:
        with (
            tc.tile_pool(name="const", bufs=1) as cpool,
            tc.tile_pool(name="work", bufs=2) as pool,
            tc.tile_pool(name="gath", bufs=2) as gpool,
            tc.tile_pool(name="stage", bufs=2) as spool,
            tc.tile_pool(name="ps", bufs=2, space="PSUM") as pspool,
            tc.tile_pool(name="psg", bufs=1, space="PSUM") as ps1pool,
        ):
            # ------------- constants -------------
            def touch(*producers):
                # PE nop that absorbs a producer's sem wait so matmuls
                # carry at most one sync-wait (codegen LW-struct limit).
                for prod in producers:
                    if prod is None:
                        continue
                    n = nc.tensor.nop(nofuse=True, hint="wait_absorb")
                    add_dep_helper(n.ins, prod.ins, sync=True,
                                   reason="pe wait absorb")

            _const_loads = []

            def load_const(name, param, shape, sl=None):
                t = cpool.tile(shape, F32, tag=name)
                i = nc.sync.dma_start(out=t[:], in_=param if sl is None else sl)
                _const_loads.append(i)
                return t

            ident = load_const("ident", identp[:], [128, 128])
            consts = load_const("consts", constp[:], [1, 3 * 128])
            W1ext_sb = load_const("w1e", W1extp[:], [FEAT + 2, HEADS * HID])
            att2_sb = load_const("att2", att2p[:], [HID, 2])
            wsa1e_sb = load_const("wsa1e", wsa1ep[:], [FEAT + 1, NSUB])
            wsa2e_sb = load_const("wsa2e", wsa2ep[:], [NSUB + 1, 1])
            Wp1e_sb = load_const("wp1e", Wp1ep[:], [HID + 1, HID // 2])
            Wp2_sb = load_const("wp2", Wp2p[:], [HID // 2, 1])
            bp2_sb = load_const("bp2", bp2p[:], [1, 1])
            b2row_sb = load_const("b2row", b2rowp[:], [1, HID])
            A1a_sb = load_const("a1a", A1p[0:128, :], [128, 2 * HEADS])
            A1b_sb = load_const("a1b", A1p[128:256, :], [128, 2 * HEADS])
            W1Ta_sb = load_const("w1ta", W1Tp[0:128, :], [128, FEAT + 1])
            W1Tb_sb = load_const("w1tb", W1Tp[128:256, :], [128, FEAT + 1])

            onecol = cpool.tile([128, 1], F32, tag="onecol")
            nc.vector.memset(onecol[:], 1.0)
            onesrow = cpool.tile([1, 128], F32, tag="onesrow")
            nc.vector.memset(onesrow[:], 1.0)
            touch(*_const_loads)

            # materialized row-broadcast constants via K=1 matmul
            def bcast_row(row_ap, n, tag, parts=128):
                ps = pspool.tile([parts, n], F32, tag="proj")
                nc.tensor.matmul(ps[:], lhsT=onesrow[:, 0:parts], rhs=row_ap,
                                 start=True, stop=True)
                t = cpool.tile([parts, n], F32, tag=tag)
                nc.scalar.copy(out=t[:], in_=ps[:])
                return t

            iotamodb = bcast_row(consts[:, 0:128], 128, "iotamodb")
            iota128b = bcast_row(consts[:, 128:256], 128, "iota128b")
            iota32b = bcast_row(consts[:, 256:256 + NSUB], NSUB, "iota32b")
            b2rowb = bcast_row(b2row_sb[:], HID, "b2rowb", NG)

            # W2e_h [64, 66] = [W2 rows h | v2 rows h] per head
            w2eh = []
            for h in range(HEADS):
                t = cpool.tile([HID, HID + 2], F32, tag=f"w2e{h}")
                i = nc.sync.dma_start(out=t[:, 0:HID],
                                      in_=W2p[h * HID:(h + 1) * HID, :])
                touch(i)
                w2eh.append(t)
            # v2 = W2 @ att2 per 128-row chunk, then SBUF->SBUF DMA into w2eh
            for ci in range(2):
                wchunk = cpool.tile([128, HID], F32, tag=f"w2c{ci}")
                touch(nc.sync.dma_start(out=wchunk[:],
                                        in_=W2p[ci * 128:(ci + 1) * 128, :]))
                pstr = pspool.tile([HID, 128], F32, tag="tr")
                nc.tensor.transpose(out=pstr[:], in_=wchunk[:],
                                    identity=ident[:])
                w2ct = pool.tile([HID, 128], F32, tag="w2ct")
                nc.scalar.copy(out=w2ct[:], in_=pstr[:])
                psv = pspool.tile([128, 2], F32, tag="proj")
                nc.tensor.matmul(psv[:], lhsT=w2ct[:], rhs=att2_sb[:],
                                 start=True, stop=True)
                v2c = pool.tile([128, 2], F32, tag="v2c")
                nc.scalar.copy(out=v2c[:], in_=psv[:])
                for hh in range(2):
                    h = ci * 2 + hh
                    touch(nc.gpsimd.dma_start(
                        out=w2eh[h][:, HID:HID + 2],
                        in_=v2c[hh * HID:(hh + 1) * HID, :]))

            # negated colsum of [W2 | v2] -> ncs [1, 66]
            psc = pspool.tile([1, HID + 2], F32, tag="proj")
            for h in range(HEADS):
                nc.tensor.matmul(psc[:], lhsT=onecol[0:HID, :], rhs=w2eh[h][:],
                                 start=(h == 0), stop=(h == HEADS - 1))
            ncs_sb = cpool.tile([1, HID + 2], F32, tag="ncs")
            nc.vector.tensor_scalar_mul(ncs_sb[:], psc[:], -1.0)

            # ------------- phase A: substructure mean + softmax weights ------
            psA = ps1pool.tile([NSUB, FEAT + 1], F32, tag="glob")
            for t in range(NNT):
                xt = pool.tile([128, FEAT], F32, tag="xt")
                touch(nc.sync.dma_start(out=xt[:],
                                        in_=xs[t * 128:(t + 1) * 128, :]))
                S = pool.tile([128, NSUB], F32, tag="S")
                nc.vector.tensor_tensor(
                    out=S[:], in0=iota32b[:],
                    in1=xt[:, 5:6].to_broadcast([128, NSUB]), op=AX.is_equal)
                nc.tensor.matmul(psA[:, 0:1], lhsT=S[:], rhs=onecol[:],
                                 start=(t == 0), stop=(t == NNT - 1))
                nc.tensor.matmul(psA[:, 1:FEAT + 1], lhsT=S[:], rhs=xt[:],
                                 start=(t == 0), stop=(t == NNT - 1))
            ssum_sb = pool.tile([NSUB, FEAT + 1], F32, tag="ssum")
            nc.scalar.copy(out=ssum_sb[:], in_=psA[:])
            nc.sync.dma_start(out=ssum_l[:], in_=ssum_sb[:])
            nc.gpsimd.collective_compute(
                "AllReduce", AX.add, replica_groups=RG,
                ins=[ssum_l[:]], outs=[ssum_g[:]])
            sums_sb = pool.tile([NSUB, FEAT + 1], F32, tag="sums")
            nc.sync.dma_start(out=sums_sb[:], in_=ssum_g[:])
            if dbg:
                nc.sync.dma_start(out=dbg_ssum[:], in_=sums_sb[:])

            cnt = pool.tile([NSUB, 1], F32, tag="cnt")
            nc.vector.tensor_scalar_max(cnt[:], sums_sb[:, 0:1], 1.0)
            recA = pool.tile([NSUB, 1], F32, tag="recA")
            nc.vector.reciprocal(recA[:], cnt[:])
            smean = pool.tile([NSUB, FEAT], F32, tag="smean")
            nc.vector.tensor_scalar_mul(smean[:], sums_sb[:, 1:FEAT + 1],
                                        recA[:])
            smeanTe = pool.tile([FEAT + 1, NSUB], F32, tag="smeanTe")
            nc.vector.memset(smeanTe[FEAT:FEAT + 1, :], 1.0)
            pstm = pspool.tile([FEAT, NSUB], F32, tag="tr")
            nc.tensor.transpose(out=pstm[:], in_=smean[:],
                                identity=ident[0:NSUB, 0:NSUB])
            nc.scalar.copy(out=smeanTe[0:FEAT, :], in_=pstm[:])
            psz = pspool.tile([NSUB, NSUB], F32, tag="proj")
            nc.tensor.matmul(psz[:], lhsT=wsa1e_sb[:], rhs=smeanTe[:],
                             start=True, stop=True)
            zAe = pool.tile([NSUB + 1, NSUB], F32, tag="zAe")
            nc.vector.memset(zAe[NSUB:NSUB + 1, :], 1.0)
            zraw = pool.tile([NSUB, NSUB], F32, tag="zraw")
            nc.scalar.copy(out=zraw[:], in_=psz[:])
            nc.vector.scalar_tensor_tensor(
                out=zAe[0:NSUB, :], in0=zraw[:], scalar=0.2, in1=zraw[:],
                op0=AX.mult, op1=AX.max)
            psl = pspool.tile([1, NSUB], F32, tag="proj")
            nc.tensor.matmul(psl[:], lhsT=wsa2e_sb[:], rhs=zAe[:],
                             start=True, stop=True)
            lmax = pool.tile([1, 1], F32, tag="lmax")
            nc.vector.tensor_reduce(lmax[:], psl[:], axis=mybir.AxisListType.X,
                                    op=AX.max)
            nlmax = pool.tile([1, 1], F32, tag="nlmax")
            nc.vector.tensor_scalar_mul(nlmax[:], lmax[:], -1.0)
            exps = pool.tile([1, NSUB], F32, tag="exps")
            sume = pool.tile([1, 1], F32, tag="sume")
            nc.scalar.activation(out=exps[:], in_=psl[:], func=AF.Exp,
                                 bias=nlmax[:], accum_out=sume[:])
            recS = pool.tile([1, 1], F32, tag="recS")
            nc.vector.reciprocal(recS[:], sume[:])
            wrow = cpool.tile([1, NSUB], F32, tag="wrow")
            nc.vector.tensor_scalar_mul(wrow[:], exps[:], recS[:])
            wrowb = bcast_row(wrow[:], NSUB, "wrowb")
            if dbg:
                nc.sync.dma_start(out=dbg_wrow[:], in_=wrow[:])

            # ------------- phase B: U and small1 table -------------
            psU = pspool.tile([FEAT + 1, 2 * HEADS], F32, tag="proj")
            nc.tensor.matmul(psU[:], lhsT=W1Ta_sb[:], rhs=A1a_sb[:],
                             start=True, stop=False)
            nc.tensor.matmul(psU[:], lhsT=W1Tb_sb[:], rhs=A1b_sb[:],
                             start=False, stop=True)
            U_sb = cpool.tile([FEAT + 1, 2 * HEADS], F32, tag="U")
            nc.scalar.copy(out=U_sb[:], in_=psU[:])
            U64 = cpool.tile([1, 2 * HEADS], F32, tag="U64")
            touch(nc.gpsimd.dma_start(out=U64[:], in_=U_sb[FEAT:FEAT + 1, :]))
            U64b = bcast_row(U64[:], 2 * HEADS, "U64b")

            for t in range(NNT):
                xt2 = pool.tile([128, FEAT], F32, tag="xt2")
                nc.sync.dma_start(out=xt2[:], in_=xs[t * 128:(t + 1) * 128, :])
                xTt = pool.tile([FEAT, 128], F32, tag="xTt")
                touch(nc.sync.dma_start(out=xTt[:],
                                        in_=xTs[:, t * 128:(t + 1) * 128]))
                asm = pool.tile([128, BGW], F32, tag="asm")
                scr = pool.tile([128, NSUB], F32, tag="scr")
                nc.vector.tensor_copy(asm[:, 0:FEAT], xt2[:])
                nc.vector.scalar_tensor_tensor(
                    out=scr[:], in0=iota32b[:], scalar=xt2[:, 5:6],
                    in1=wrowb[:], op0=AX.is_equal, op1=AX.mult,
                    accum_out=asm[:, FEAT:FEAT + 1])
                nc.vector.memset(asm[:, FEAT + 1:FEAT + 2], 1.0)
                nc.vector.memset(asm[:, FEAT + 10:BGW], 0.0)
                psB = pspool.tile([128, 2 * HEADS], F32, tag="proj")
                nc.tensor.matmul(psB[:], lhsT=xTt[:], rhs=U_sb[0:FEAT, :],
                                 start=True, stop=True)
                t2 = pool.tile([128, 2 * HEADS], F32, tag="t2")
                nc.vector.tensor_tensor(
                    out=t2[:],
                    in0=asm[:, FEAT:FEAT + 1].to_broadcast([128, 2 * HEADS]),
                    in1=U64b[:], op=AX.mult)
                nc.vector.tensor_tensor(out=asm[:, FEAT + 2:FEAT + 10],
                                        in0=psB[:], in1=t2[:], op=AX.add)
                nc.sync.dma_start(out=big1_l[t * 128:(t + 1) * 128, :],
                                  in_=asm[:])
            nc.gpsimd.collective_compute(
                "AllGather", AX.bypass, replica_groups=RG,
                ins=[big1_l[:]], outs=[big1[:]])
            if dbg:
                nc.sync.dma_start(out=dbg_small[0:128, :],
                                  in_=big1[0:128, FEAT:FEAT + 10])
                nc.sync.dma_start(out=dbg_small[128:256, :],
                                  in_=big1[3 * NSHP:3 * NSHP + 128,
                                           FEAT:FEAT + 10])

            # ------------- phase C: layer 1 -------------
            def l1_chunk_finish(stg, ci):
                s_sb = spool.tile([HID, HEADS * 128], F32, tag="s_sb")
                nc.scalar.activation(out=s_sb[:], in_=stg[:], func=AF.Relu,
                                     scale=-1.0)
                u_sb = spool.tile([HID, HEADS * 128], F32, tag="u_sb")
                nc.scalar.activation(out=u_sb[:], in_=s_sb[:], func=AF.Exp,
                                     scale=-1.0)
                p_sb = spool.tile([HID, HEADS * 128], F32, tag="p_sb")
                nc.vector.tensor_scalar_max(p_sb[:], stg[:], 0.0)
                psM = pspool.tile([HID + 2, 128], F32, tag="proj")
                for h in range(HEADS):
                    nc.tensor.matmul(
                        psM[:], lhsT=w2eh[h][:],
                        rhs=p_sb[:, h * 128:(h + 1) * 128],
                        start=(h == 0), stop=False)
                    nc.tensor.matmul(
                        psM[:], lhsT=w2eh[h][:],
                        rhs=u_sb[:, h * 128:(h + 1) * 128],
                        start=False, stop=False)
                nc.tensor.matmul(psM[:], lhsT=ncs_sb[:], rhs=onesrow[:],
                                 start=False, stop=True)
                mT_sb = spool.tile([HID + 2, 128], F32, tag="mT_sb")
                nc.scalar.copy(out=mT_sb[:], in_=psM[:])
                psX = pspool.tile([128, HID + 2], F32, tag="tr")
                nc.tensor.transpose(out=psX[:], in_=mT_sb[:],
                                    identity=ident[0:HID + 2, 0:HID + 2])
                xrow = spool.tile([128, XCW], F32, tag="xrow")
                nc.scalar.copy(out=xrow[:, 0:HID], in_=psX[:, 0:HID])
                nc.scalar.copy(out=xrow[:, HID + 1:HID + 3],
                               in_=psX[:, HID:HID + 2])
                nc.vector.memset(xrow[:, HID:HID + 1], 1.0)
                nc.vector.memset(xrow[:, HID + 3:XCW], 0.0)
                nc.sync.dma_start(
                    out=xchg_l[ci * 128:(ci + 1) * 128, :], in_=xrow[:])

            t_global = 0
            stg = None
            batch_tiles = {}
            for w in range(NW1):
                nt = tiles1[w]
                psW = pspool.tile([128, FEAT + 2], F32, tag="accum")
                for k in range(nt):
                    t = t_global + k
                    b, j = divmod(t, GB)
                    if j == 0:
                        nbt = min(GB, T1 - b * GB)
                        ix1 = gpool.tile([128, GB], I32, tag="ix1")
                        nc.sync.dma_start(out=ix1[:, 0:nbt],
                                          in_=e1s[b, :, 0:nbt])
                        ix1d = gpool.tile([128, GB], I32, tag="ix1d")
                        nc.sync.dma_start(out=ix1d[:, 0:nbt],
                                          in_=e1d[b, :, 0:nbt])
                        do1 = gpool.tile([128, GB], F32, tag="do1")
                        nc.sync.dma_start(out=do1[:, 0:nbt],
                                          in_=e1o[b, :, 0:nbt])
                        grh = gpool.tile([128, GB, RHW], F32, tag="grh")
                        gd = gpool.tile([128, GB, HEADS], F32, tag="gd")
                        big1f = big1[:].rearrange("n a -> (n a) ()")
                        for jj in range(nbt):
                            gij = nc.gpsimd.indirect_dma_start(
                                out=grh[:, jj, :], out_offset=None,
                                in_=big1[:],
                                in_offset=bass.IndirectOffsetOnAxis(
                                    ap=ix1[:, jj:jj + 1], axis=0))
                            if jj % 8 == 0:
                                touch(gij)
                            nc.gpsimd.indirect_dma_start(
                                out=gd[:, jj, :], out_offset=None,
                                in_=big1f,
                                in_offset=bass.IndirectOffsetOnAxis(
                                    ap=ix1d[:, jj:jj + 1], axis=0))
                        exB = gpool.tile([128, GB, HEADS], F32, tag="exB")
                        nc.vector.tensor_tensor(
                            out=exB[:, 0:nbt, :],
                            in0=grh[:, 0:nbt, FEAT + 2:FEAT + 6],
                            in1=gd[:, 0:nbt, :], op=AX.add)
                        nc.vector.scalar_tensor_tensor(
                            out=exB[:, 0:nbt, :], in0=exB[:, 0:nbt, :],
                            scalar=0.2, in1=exB[:, 0:nbt, :],
                            op0=AX.mult, op1=AX.max)
                        if dbg and b == 0:
                            nc.sync.dma_start(
                                out=dbg_gd[:],
                                in_=gd[:].rearrange("p a b -> p (a b)"))
                            nc.sync.dma_start(
                                out=dbg_e4[:],
                                in_=exB[:].rearrange("p a b -> p (a b)"))
                        nc.scalar.activation(out=exB[:, 0:nbt, :],
                                             in_=exB[:, 0:nbt, :],
                                             func=AF.Exp)
                        if dbg and b == 0:
                            nc.sync.dma_start(
                                out=dbg_exb[:],
                                in_=exB[:].rearrange("p a b -> p (a b)"))
                            nc.sync.dma_start(
                                out=dbg_grh[:],
                                in_=grh[:].rearrange("p a b -> p (a b)"))
                        batch_tiles = dict(ix1=ix1, grh=grh, do1=do1, exB=exB)
                    grh, do1, exB = (batch_tiles["grh"], batch_tiles["do1"],
                                     batch_tiles["exB"])
                    M4 = pool.tile([128, 128], F32, tag="M4")
                    nc.vector.scalar_tensor_tensor(
                        out=M4[:].rearrange("p (h d) -> p h d", h=HEADS),
                        in0=iotamodb[:].rearrange("p (h d) -> p h d", h=HEADS),
                        scalar=do1[:, j:j + 1],
                        in1=exB[:, j, :].unsqueeze(2).to_broadcast(
                            [128, HEADS, W1WIN]),
                        op0=AX.is_equal, op1=AX.mult)
                    nc.tensor.matmul(psW[:], lhsT=M4[:],
                                     rhs=grh[:, j, 0:FEAT + 2],
                                     start=(k == 0), stop=(k == nt - 1))
                t_global += nt

                recW = pool.tile([128, 1], F32, tag="recW")
                if dbg and w == 0:
                    pswd = pool.tile([128, FEAT + 2], F32, tag="pswd")
                    nc.vector.tensor_copy(pswd[:], psW[:])
                    nc.sync.dma_start(out=dbg_psw[:], in_=pswd[:])
                nc.vector.reciprocal(recW[:], psW[:, FEAT + 1:FEAT + 2])
                agg = pool.tile([128, FEAT + 2], F32, tag="agg")
                nc.vector.tensor_scalar_mul(agg[:], psW[:], recW[:])
                if dbg and w == 0:
                    nc.sync.dma_start(out=dbg_agg[:], in_=agg[:])
                psT2 = pspool.tile([FEAT + 2, 128], F32, tag="tr")
                nc.tensor.transpose(out=psT2[:], in_=agg[:], identity=ident[:])
                aggT = pool.tile([FEAT + 2, 128], F32, tag="aggT")
                nc.scalar.copy(out=aggT[:], in_=psT2[:])

                ci, wi = divmod(w, 4)
                if wi == 0:
                    stg = spool.tile([HID, HEADS * 128], F32, tag="stg")
                psP = pspool.tile([HID, 128], F32, tag="proj")
                for h in range(HEADS):
                    nc.tensor.matmul(
                        psP[:, h * W1WIN:(h + 1) * W1WIN],
                        lhsT=W1ext_sb[:, h * HID:(h + 1) * HID],
                        rhs=aggT[:, h * W1WIN:(h + 1) * W1WIN],
                        start=True, stop=True)
                nc.scalar.copy(
                    out=stg[:].rearrange("p (h d) -> p h d", h=HEADS)
                        [:, :, wi * W1WIN:(wi + 1) * W1WIN],
                    in_=psP[:].rearrange("p (h d) -> p h d", h=HEADS))
                if wi == 3:
                    if dbg and ci == 0:
                        nc.sync.dma_start(out=dbg_stg[:], in_=stg[:])
                    l1_chunk_finish(stg, ci)

            nc.gpsimd.collective_compute(
                "AllGather", AX.bypass, replica_groups=RG,
                ins=[xchg_l[:]], outs=[xchg[:]])
            if dbg:
                nc.sync.dma_start(out=dbg_xchg[0:128, :], in_=xchg[0:128, :])
                nc.sync.dma_start(out=dbg_xchg[128:256, :],
                                  in_=xchg[5 * NSHP:5 * NSHP + 128, :])

            # ------------- phase D: layer 2 + pooling -------------
            psG = ps1pool.tile([NG, HID + 1], F32, tag="glob")
            t_global = 0
            batch_tiles2 = {}
            for w in range(NW2):
                nt = tiles2[w]
                psW2 = pspool.tile([128, HID + 1], F32, tag="accum")
                for k in range(nt):
                    t = t_global + k
                    b, j = divmod(t, GB)
                    if j == 0:
                        nbt = min(GB, T2 - b * GB)
                        ix2 = gpool.tile([128, GB], I32, tag="ix2")
                        nc.sync.dma_start(out=ix2[:, 0:nbt],
                                          in_=e2s[b, :, 0:nbt])
                        ix2d = gpool.tile([128, GB], I32, tag="ix2d")
                        nc.sync.dma_start(out=ix2d[:, 0:nbt],
                                          in_=e2d[b, :, 0:nbt])
                        do2 = gpool.tile([128, GB], F32, tag="do2")
                        nc.sync.dma_start(out=do2[:, 0:nbt],
                                          in_=e2o[b, :, 0:nbt])
                        gm = gpool.tile([128, GB, XCW], F32, tag="gm")
                        gd2 = gpool.tile([128, GB, 1], F32, tag="gd2")
                        xchgf = xchg[:].rearrange("n a -> (n a) ()")
                        for jj in range(nbt):
                            gij = nc.gpsimd.indirect_dma_start(
                                out=gm[:, jj, :], out_offset=None, in_=xchg[:],
                                in_offset=bass.IndirectOffsetOnAxis(
                                    ap=ix2[:, jj:jj + 1], axis=0))
                            if jj % 8 == 0:
                                touch(gij)
                            nc.gpsimd.indirect_dma_start(
                                out=gd2[:, jj, :], out_offset=None,
                                in_=xchgf,
                                in_offset=bass.IndirectOffsetOnAxis(
                                    ap=ix2d[:, jj:jj + 1], axis=0))
                        ex2B = gpool.tile([128, GB, 1], F32, tag="ex2B")
                        nc.vector.tensor_tensor(
                            out=ex2B[:, 0:nbt, :],
                            in0=gm[:, 0:nbt, HID + 1:HID + 2],
                            in1=gd2[:, 0:nbt, :], op=AX.add)
                        nc.vector.scalar_tensor_tensor(
                            out=ex2B[:, 0:nbt, :], in0=ex2B[:, 0:nbt, :],
                            scalar=0.2, in1=ex2B[:, 0:nbt, :],
                            op0=AX.mult, op1=AX.max)
                        nc.scalar.activation(out=ex2B[:, 0:nbt, :],
                                             in_=ex2B[:, 0:nbt, :],
                                             func=AF.Exp)
                        batch_tiles2 = dict(gm=gm, do2=do2, ex2B=ex2B)
                    gm, do2, ex2B = (batch_tiles2["gm"], batch_tiles2["do2"],
                                     batch_tiles2["ex2B"])
                    M1 = pool.tile([128, 128], F32, tag="M1")
                    nc.vector.scalar_tensor_tensor(
                        out=M1[:], in0=iota128b[:], scalar=do2[:, j:j + 1],
                        in1=ex2B[:, j, :].to_broadcast([128, 128]),
                        op0=AX.is_equal, op1=AX.mult)
                    nc.tensor.matmul(psW2[:], lhsT=M1[:],
                                     rhs=gm[:, j, 0:HID + 1],
                                     start=(k == 0), stop=(k == nt - 1))
                t_global += nt

                recW2 = pool.tile([128, 1], F32, tag="recW2")
                nc.vector.reciprocal(recW2[:], psW2[:, HID:HID + 1])
                h2 = pool.tile([128, HID + 1], F32, tag="h2")
                nc.vector.memset(h2[:, 0:1], 1.0)
                nc.vector.tensor_scalar_mul(h2[:, 1:HID + 1],
                                            psW2[:, 0:HID], recW2[:])
                bo = pool.tile([128, 1], F32, tag="bo")
                nc.sync.dma_start(out=bo[:], in_=boffp[w:w + 1, :]
                                  .rearrange("a b -> b a"))
                B = pool.tile([128, NG], F32, tag="B")
                nc.vector.tensor_tensor(
                    out=B[:], in0=iota128b[:, 0:NG],
                    in1=bo[:].to_broadcast([128, NG]), op=AX.is_equal)
                nc.tensor.matmul(psG[:], lhsT=B[:], rhs=h2[:],
                                 start=(w == 0), stop=(w == NW2 - 1))

            gsb = pool.tile([NG, HID + 1], F32, tag="gsb")
            nc.scalar.copy(out=gsb[:], in_=psG[:])
            nc.sync.dma_start(out=g_l[:], in_=gsb[:])
            nc.gpsimd.collective_compute(
                "AllReduce", AX.add, replica_groups=RG,
                ins=[g_l[:]], outs=[g_g[:]])
            g2 = pool.tile([NG, HID + 1], F32, tag="g2")
            nc.sync.dma_start(out=g2[:], in_=g_g[:])
            if dbg:
                nc.sync.dma_start(out=dbg_g[:], in_=g2[:])

            # ------------- phase E: head MLP -------------
            h2g = pool.tile([NG, HID], F32, tag="h2g")
            nc.vector.scalar_tensor_tensor(
                out=h2g[:], in0=b2rowb[0:NG, :], scalar=g2[:, 0:1],
                in1=g2[:, 1:HID + 1], op0=AX.mult, op1=AX.add)
            psHT = pspool.tile([HID, NG], F32, tag="tr")
            nc.tensor.transpose(out=psHT[:], in_=h2g[:], identity=ident[:])
            gTe = pool.tile([HID + 1, NG], F32, tag="gTe")
            nc.vector.memset(gTe[HID:HID + 1, :], 1.0)
            nc.scalar.copy(out=gTe[0:HID, :], in_=psHT[:])
            psZ = pspool.tile([HID // 2, NG], F32, tag="proj")
            nc.tensor.matmul(psZ[:], lhsT=Wp1e_sb[:], rhs=gTe[:],
                             start=True, stop=True)
            pz = pool.tile([HID // 2, NG], F32, tag="pz")
            nc.vector.tensor_scalar_max(pz[:], psZ[:], 0.0)
            sz = pool.tile([HID // 2, NG], F32, tag="sz")
            nc.scalar.activation(out=sz[:], in_=psZ[:], func=AF.Relu,
                                 scale=-1.0)
            uz = pool.tile([HID // 2, NG], F32, tag="uz")
            nc.scalar.activation(out=uz[:], in_=sz[:], func=AF.Exp,
                                 scale=-1.0)
            pscp = pspool.tile([1, 1], F32, tag="proj")
            nc.tensor.matmul(pscp[:], lhsT=onecol[0:HID // 2, :],
                             rhs=Wp2_sb[:], start=True, stop=True)
            cF = pool.tile([1, 1], F32, tag="cF")
            nc.vector.tensor_scalar(cF[:], pscp[:], -1.0, bp2_sb[:],
                                    op0=AX.mult, op1=AX.add)
            psF = pspool.tile([1, NG], F32, tag="proj")
            nc.tensor.matmul(psF[:], lhsT=Wp2_sb[:], rhs=pz[:],
                             start=True, stop=False)
            nc.tensor.matmul(psF[:], lhsT=Wp2_sb[:], rhs=uz[:],
                             start=False, stop=False)
            nc.tensor.matmul(psF[:], lhsT=cF[:], rhs=onesrow[:, 0:NG],
                             start=False, stop=True)
            ores = pool.tile([1, NG], F32, tag="ores")
            nc.scalar.copy(out=ores[:], in_=psF[:])
            nc.sync.dma_start(out=outp[:].rearrange("a b -> b a"),
                              in_=ores[:])

    return nc


# ----------------------------------------------------------------------------
# Entry point
# ----------------------------------------------------------------------------

def make_in_maps(inputs):
    x = np.asarray(inputs["x"], np.float32)
    edge_index = np.asarray(inputs["edge_index"], np.int64)
    batch = np.asarray(inputs["batch"], np.int64)

    cfg = _prep(x, edge_index, batch)
    wp = _weights_pack(inputs)

    consts = np.zeros((1, 3 * 128), np.float32)
    consts[0, 0:128] = np.tile(np.arange(W1WIN, dtype=np.float32), HEADS)
    consts[0, 128:256] = np.arange(128, dtype=np.float32)
    consts[0, 256:256 + NSUB] = np.arange(NSUB, dtype=np.float32)

    shared = dict(ident=np.eye(128, dtype=np.float32),
                  consts=consts, **wp)
    in_maps = []
    for c in range(NC):
        m = dict(shared)
        m.update(xs=cfg["xs"][c], xTs=cfg["xTs"][c],
                 e1s=cfg["b1s"][c], e1d=cfg["b1d"][c], e1o=cfg["b1o"][c],
                 e2s=cfg["b2s"][c], e2d=cfg["b2d"][c], e2o=cfg["b2o"][c],
                 boff=cfg["boff"][c])
        in_maps.append(m)
    return cfg, in_maps


_NOSPLIT = None


def _split_matmul_waits(nc):
    """Walrus codegen allows only one sync-wait on most engine instruction
    structs; move extra waits onto same-engine no-ops inserted right before,
    one wait each."""
    global _NOSPLIT
    if _NOSPLIT is None:
        _NOSPLIT = (mybir.InstEventSemaphore, mybir.InstAllEngineBarrier,
                    mybir.InstUnconditionalBranch, mybir.InstCompareAndBranch,
                    mybir.InstIndirectBranch, mybir.InstBranchHint,
                    mybir.InstNoOp, mybir.InstHalt)
    nsplit = 0
    for fn in nc.m.functions:
        for bb in fn.blocks:
            il = bb.instructions
            out = []
            for ins in il:
                si = ins.sync_info
                if (not isinstance(ins, _NOSPLIT) and ins.engine is not None
                        and si is not None and si.on_wait
                        and len(si.on_wait) > 1):
                    waits = list(si.on_wait)
                    for k, wt in enumerate(waits[:-1]):
                        nop = mybir.InstNoOp(
                            name=f"{ins.name}-ws{k}", ins=[], outs=[])
                        nop.engine = ins.engine
                        nop.sync_info = mybir.SyncInfo(
                            on_wait=[wt], on_update=[])
                        out.append(nop)
                    si.on_wait = waits[-1:]
                    nsplit += 1
                out.append(ins)
            il[:] = out
    return nsplit


def kernel(**inputs):
    from concourse.bass_utils import run_bass_kernel_spmd

    cfg, in_maps = make_in_maps(inputs)
    nc = _build(cfg)
    _split_matmul_waits(nc)
    res = run_bass_kernel_spmd(nc, in_maps, list(range(NC)))
    return np.asarray(res.results[0]["out"], np.float32)

